# revision 6
# baseline (speedup 1.0000x reference)
"""Trainium2 Bass kernel for nn_ContrastiveLoss (NT-Xent-style loss with
tag/document masking).

Strategy (8 NeuronCores, SPMD):
  - Rows of the 8192x8192 similarity matrix are sharded: core c owns 1024 rows.
  - Each core receives the full concatenated embedding matrix TRANSPOSED
    ([256, 8192] fp32) with its columns ROLLED so the core's own 1024 rows sit
    at columns [0:1024].  This makes the program identical on every core (pure
    SPMD, no partition-id control flow): lhsT is always columns [0:1024] and
    the positive-pair partner of local row m is always column 4096+m.
  - On device: L2-normalize (squares -> ones-matmul partition reduction ->
    1/sqrt -> column-scale), cast to bf16, then for each 128-row tile compute
    sim = repsT.T @ repsT in [128,512] PSUM chunks, exp(2*sim) on ACT,
    tag/doc not-equal masks on DVE/GPSIMD (fp16), masked row-sums via
    scalar_tensor_tensor accum, per-row loss = ln(denom+0.1) - 2*sim_diag.
  - Each core DMAs a [128, 8] per-row-partial tensor out; the host sums all
    partials and divides by 2B.
"""

import sys

for _p in ("/opt/trn_rl_repo", "/root/.axon_site/_ro/trn_rl_repo"):
    if _p not in sys.path:
        sys.path.insert(0, _p)

from contextlib import ExitStack

import numpy as np

from concourse import bacc, mybir, tile
from concourse.bass_utils import run_bass_kernel_spmd

F32 = mybir.dt.float32
F16 = mybir.dt.float16
BF16 = mybir.dt.bfloat16

P = 128          # SBUF partitions
B = 4096         # batch
D = 256          # embedding dim
N = 2 * B        # 8192 rows/cols of the similarity matrix
CORES = 8
ROWS_PER_CORE = N // CORES      # 1024
NI = ROWS_PER_CORE // P         # 8 row tiles per core
CH = 512                        # column chunk (one PSUM bank of fp32)
NJ = N // CH                    # 16 column chunks
KT = D // P                     # 2 contraction tiles
TEMP_SCALE = 2.0                # 1 / TEMPERATURE


def _build_program():
    nc = bacc.Bacc(None, target_bir_lowering=False)

    embT_d = nc.declare_dram_parameter("embT", [D, N], F32, isOutput=False)
    tagsv_d = nc.declare_dram_parameter("tagsv", [1, N], F16, isOutput=False)
    docsv_d = nc.declare_dram_parameter("docsv", [1, N], F16, isOutput=False)
    tagst_d = nc.declare_dram_parameter("tagst", [P, NI], F32, isOutput=False)
    docst_d = nc.declare_dram_parameter("docst", [P, NI], F32, isOutput=False)
    ident_d = nc.declare_dram_parameter("ident", [P, P], F16, isOutput=False)
    out_d = nc.declare_dram_parameter("out", [P, NI], F32, isOutput=True)

    Exp = mybir.ActivationFunctionType.Exp
    Ln = mybir.ActivationFunctionType.Ln
    Square = mybir.ActivationFunctionType.Square
    Sqrt = mybir.ActivationFunctionType.Sqrt
    mult = mybir.AluOpType.mult
    add = mybir.AluOpType.add
    not_equal = mybir.AluOpType.not_equal

    with tile.TileContext(nc) as tc, ExitStack() as ctx:
        persist = ctx.enter_context(tc.tile_pool(name="persist", bufs=1))
        repsT = [persist.tile([P, N], BF16, tag=f"repsT{k}", name=f"repsT{k}") for k in range(KT)]
        tags_b = persist.tile([P, N], F16, tag="tags_b")
        docs_b = persist.tile([P, N], F16, tag="docs_b")
        rn16 = persist.tile([1, N], F16, tag="rn16")
        rn_b = persist.tile([P, N], F16, tag="rn_b")
        tagst = persist.tile([P, NI], F32, tag="tagst")
        docst = persist.tile([P, NI], F32, tag="docst")
        ident = persist.tile([P, P], F16, tag="ident")
        v_sb = persist.tile([P, NI], F32, tag="v_sb")
        ones = persist.tile([P, 1], F16, tag="ones")
        pt1 = persist.tile([P, 1], F32, tag="pt1")

        nc.sync.dma_start(tagst[:], tagst_d[:])
        nc.sync.dma_start(docst[:], docst_d[:])
        nc.sync.dma_start(ident[:], ident_d[:])
        nc.vector.memset(ones[:], 1.0)
        nc.vector.memset(pt1[:], 0.1)

        # ---- Phase A: load embT, compute 1/||row|| per column -------------
        with (
            tc.tile_pool(name="embp", bufs=1) as embp,
            tc.tile_pool(name="small", bufs=3) as smallp,
            tc.tile_pool(name="psn", bufs=4, space="PSUM") as psn,
        ):
            e = [embp.tile([P, N], F32, tag=f"e{k}", name=f"e{k}") for k in range(KT)]
            for k in range(KT):
                nc.sync.dma_start(e[k][:], embT_d[k * P:(k + 1) * P, :])

            for j in range(NJ):
                js = slice(j * CH, (j + 1) * CH)
                n2 = psn.tile([1, CH], F32, tag="n2")
                for k in range(KT):
                    sq = smallp.tile([P, CH], F16, tag="sq")
                    nc.scalar.activation(sq[:], e[k][:, js], Square)
                    nc.tensor.matmul(
                        n2[:], ones[:], sq[:], start=(k == 0), stop=(k == KT - 1)
                    )
                # 1/sqrt(norm2): reciprocal on DVE then sqrt on ACT
                rcp = smallp.tile([1, CH], F32, tag="rcp")
                nc.vector.reciprocal(rcp[:], n2[:])
                nc.scalar.activation(rn16[:, js], rcp[:], Sqrt)

            # ---- Phase B: broadcasts (tags, docs, rnorm) ------------------
            tv = smallp.tile([1, N], F16, tag="tv", bufs=1)
            nc.sync.dma_start(tv[:], tagsv_d[:])
            nc.gpsimd.partition_broadcast(tags_b[:], tv[:])
            nc.sync.dma_start(tv[:], docsv_d[:])
            nc.gpsimd.partition_broadcast(docs_b[:], tv[:])
            nc.gpsimd.partition_broadcast(rn_b[:], rn16[:])

            # ---- Phase C: normalize + cast to bf16 ------------------------
            # repsT = embT * rnorm (column-wise), bf16 out
            for k in range(KT):
                nc.vector.scalar_tensor_tensor(
                    repsT[k][:], e[k][:], 1.0, rn_b[:], mult, mult
                )

        # ---- Phase D: main loop -------------------------------------------
        with (
            tc.tile_pool(name="work", bufs=4) as work,
            tc.tile_pool(name="acc", bufs=2) as accp,
            tc.tile_pool(name="psm", bufs=8, space="PSUM") as psm,
        ):
            for i in range(NI):
                ms = slice(i * P, (i + 1) * P)
                denom = accp.tile([P, NJ], F32, tag="denom")
                sd = accp.tile([P, 1], F32, tag="sd")
                jstar = (B + i * P) // CH
                off = (i * P) % CH

                S = [None] * NJ
                for g in range(2):
                    for k in range(KT):
                        for jj in range(NJ // 2):
                            j = g * (NJ // 2) + jj
                            js = slice(j * CH, (j + 1) * CH)
                            if k == 0:
                                S[j] = psm.tile([P, CH], F32, tag="S", name=f"S{j}")
                            nc.tensor.matmul(
                                S[j][:],
                                repsT[k][:, ms],
                                repsT[k][:, js],
                                start=(k == 0),
                                stop=(k == KT - 1),
                            )

                for j in range(NJ):
                    js = slice(j * CH, (j + 1) * CH)
                    kt_t = work.tile([P, CH], F16, tag="kt")
                    kd_t = work.tile([P, CH], F16, tag="kd")
                    keep = work.tile([P, CH], F16, tag="keep")
                    Et = work.tile([P, CH], F16, tag="Et")
                    junk = work.tile([P, CH], F16, tag="junk")

                    nc.vector.tensor_scalar(
                        kt_t[:], tags_b[:, js], tagst[:, i:i + 1], None, not_equal
                    )
                    nc.vector.tensor_scalar(
                        kd_t[:], docs_b[:, js], docst[:, i:i + 1], None, not_equal
                    )
                    nc.vector.tensor_tensor(keep[:], kt_t[:], kd_t[:], mult)
                    nc.scalar.activation(Et[:], S[j][:], Exp, scale=TEMP_SCALE)
                    if j == jstar:
                        junkd = work.tile([P, P], F16, tag="junkd")
                        nc.vector.scalar_tensor_tensor(
                            junkd[:], ident[:], 1.0, S[j][:, off:off + P],
                            mult, mult, accum_out=sd[:],
                        )
                    nc.vector.scalar_tensor_tensor(
                        junk[:], Et[:], 1.0, keep[:], mult, mult,
                        accum_out=denom[:, j:j + 1],
                    )

                # epilogue for row-tile i
                dtot = work.tile([P, 1], F32, tag="dtot")
                lg = work.tile([P, 1], F32, tag="lg")
                nc.vector.tensor_reduce(dtot[:], denom[:], mybir.AxisListType.X, add)
                nc.scalar.activation(lg[:], dtot[:], Ln, bias=pt1[:])
                nc.vector.scalar_tensor_tensor(
                    v_sb[:, i:i + 1], sd[:], -2.0, lg[:], mult, add
                )

            nc.sync.dma_start(out_d[:], v_sb[:])

    nc.compile()
    return nc


_NC_CACHE = []


def _get_nc():
    if not _NC_CACHE:
        _NC_CACHE.append(_build_program())
    return _NC_CACHE[0]


def _prepare_inputs(emb_i, emb_j, tags, document_ids):
    emb = np.concatenate([np.asarray(emb_i), np.asarray(emb_j)], axis=0)
    embT = np.ascontiguousarray(emb.T.astype(np.float32))          # [256, 8192]
    tags2 = np.concatenate([tags, tags]).astype(np.float16)        # [8192]
    docs2 = np.concatenate([document_ids, document_ids]).astype(np.float16)
    ident = np.eye(P, dtype=np.float16)

    in_maps = []
    for c in range(CORES):
        r = c * ROWS_PER_CORE
        roll = np.r_[r:N, 0:r]
        tv = tags2[roll]
        dv = docs2[roll]
        in_maps.append({
            "embT": np.ascontiguousarray(embT[:, roll]),
            "tagsv": tv.reshape(1, N),
            "docsv": dv.reshape(1, N),
            "tagst": np.ascontiguousarray(tv[:ROWS_PER_CORE].reshape(NI, P).T.astype(np.float32)),
            "docst": np.ascontiguousarray(dv[:ROWS_PER_CORE].reshape(NI, P).T.astype(np.float32)),
            "ident": ident,
        })
    return in_maps


def kernel(emb_i, emb_j, tags, num_classes, document_ids):
    nc = _get_nc()
    in_maps = _prepare_inputs(emb_i, emb_j, tags, document_ids)
    res = run_bass_kernel_spmd(nc, in_maps, list(range(CORES)))
    total = 0.0
    for c in range(CORES):
        total += np.asarray(res.results[c]["out"]).astype(np.float64).sum()
    return np.float32(total / N)
